# revision 15
# baseline (speedup 1.0000x reference)
"""ChainKinematics Trainium2 kernel (8-core data-parallel), v5.

Math per batch element b:
  T_curr_i = offsets[i] @ Rz(theta[b, i])
  abs_i = abs_{i-1} @ T_curr_i           (abs_{-1} = I)
  rel_i = reset_i ? T_curr_i : rel_{i-1} @ T_curr_i

Layout (per core, 8192 batch elements, fp16 state):
  State S[k*32+g, r*256+bw] = A[g*256+bw, r, k].  Every chain slot is
  r-split into two independent 512-wide sub-chains (r in {0,1} / {2,3}).
  Per sub:
    U  = wd_i^T @ S            (PE; m-blocks [u0,u1,u1,u0], PSUM f32)
    X[64:128] = w2_i^T @ S     (PE; m-blocks [u2,u3] at partition base 64)
    pq = U * trig_i            (DVE; trig q-blocks [c,c,s,-s]; fp16 SBUF)
    X[0:64] = wsum^T @ pq      (PE; block sum -> [col0, col1])
    s_next = copy(X)           (ACT; one f32 PSUM -> fp16 SBUF copy)
  The theta input ships range-reduced per q-block (cos blocks use the
  +0.25-turn-shifted reduction), so trig is a single ACT Sin per chunk
  with per-partition scale [1,1,1,-1] / bias [pi/2, pi/2, 0, 0].
  All input DMAs are issued upfront (theta chunk 0 first) so the SP DMA
  queue never head-of-line blocks chain-dependent input loads.  For dual
  bodies the rel slot (whose deps are a full body older) is issued before
  the abs slot on every engine queue.
"""

import sys

sys.path.insert(0, "/opt/trn_rl_repo")

import numpy as np

N_BODIES = 32
BATCH = 65536
N_CORES = 8
BC = BATCH // N_CORES  # 8192 per core
G = 32  # batch groups (partition blocks)
BW = BC // G  # 256 batch per group
FH = 4 * BW  # 1024: free size of one chain-slot (r, bw)
SUB = FH // 2  # 512: r-split sub-slot
PI = float(np.pi)

# trig sin chunks over the (i, bw) free dim and the body index before
# which each chunk is issued
TRIG_BOUNDS = [0, 256, 512, 1024, 2048, 3072, 4096, 5120, 6144, 7168, 8192]
TRIG_ISSUE_BODY = [0, 0, 0, 1, 3, 5, 7, 9, 11, 13]
# upfront input DMA chunks for threp
TH_DMA_BOUNDS = [0, 256, 2048, 8192]

_cache = {}


def _build_program(resets):
    from concourse import bass, mybir, tile, bacc

    f32 = mybir.dt.float32
    f16 = mybir.dt.float16

    split = resets[0] if resets else N_BODIES  # first dual body

    nc = bacc.Bacc(None, target_bir_lowering=False, debug=False)
    threp_d = nc.dram_tensor("threp", [128, BC], f32, kind="ExternalInput")
    wall_d = nc.dram_tensor("wall", [128, N_BODIES * 192], f16, kind="ExternalInput")
    wsum_d = nc.dram_tensor("wsum", [128, 64], f16, kind="ExternalInput")
    oabs_d = nc.dram_tensor("oabs", [N_BODIES, 128, FH], f16, kind="ExternalOutput")
    orel_d = nc.dram_tensor(
        "orel", [N_BODIES - split, 128, FH], f16, kind="ExternalOutput"
    )

    with tile.TileContext(nc) as tc:
        with (
            tc.tile_pool(name="wpool", bufs=1) as wpool,
            tc.tile_pool(name="trigpool", bufs=1) as trigpool,
            tc.tile_pool(name="cpool", bufs=1) as cpool,
            tc.tile_pool(name="spool", bufs=4) as spool,
            tc.tile_pool(name="idpool", bufs=1) as idpool,
            tc.tile_pool(name="pqpool", bufs=6) as pqpool,
            tc.tile_pool(name="upool", bufs=4, space=bass.MemorySpace.PSUM) as upool,
            tc.tile_pool(name="xpool", bufs=4, space=bass.MemorySpace.PSUM) as xpool,
        ):
            # ---- upfront input DMAs (no waits; body-0 deps first) ----
            threp = trigpool.tile([128, BC], f32)
            w_tile = wpool.tile([128, N_BODIES * 192], f16)
            wsum = wpool.tile([128, 64], f16)
            nc.sync.dma_start(
                threp[:, 0 : TH_DMA_BOUNDS[1]], threp_d[:, 0 : TH_DMA_BOUNDS[1]]
            )
            nc.sync.dma_start(w_tile[:, 0 : split * 192], wall_d[:, 0 : split * 192])
            nc.sync.dma_start(wsum[:], wsum_d[:])
            for lo, hi in zip(TH_DMA_BOUNDS[1:-1], TH_DMA_BOUNDS[2:]):
                nc.sync.dma_start(threp[:, lo:hi], threp_d[:, lo:hi])
            nc.sync.dma_start(w_tile[:, split * 192 :], wall_d[:, split * 192 :])

            # ---- per-partition sin constants ----
            scl = cpool.tile([128, 1], f32)  # sin scale: [1,1,1,-1]
            nc.vector.memset(scl[0:96, :], 1.0)
            nc.vector.memset(scl[96:128, :], -1.0)
            bias = cpool.tile([128, 1], f32)  # sin bias: [pi/2, pi/2, 0, 0]
            nc.vector.memset(bias[0:64, :], float(PI / 2))
            nc.vector.memset(bias[64:128, :], 0.0)

            trig = trigpool.tile([128, BC], f16)

            n_chunks = len(TRIG_BOUNDS) - 1
            next_chunk = [0]

            def issue_trig(body):
                while (
                    next_chunk[0] < n_chunks
                    and TRIG_ISSUE_BODY[next_chunk[0]] <= body
                ):
                    k = next_chunk[0]
                    lo, hi = TRIG_BOUNDS[k], TRIG_BOUNDS[k + 1]
                    nc.scalar.activation(
                        trig[:, lo:hi], threp[:, lo:hi],
                        mybir.ActivationFunctionType.Sin,
                        bias=bias[:, 0:1], scale=scl[:, 0:1],
                    )
                    next_chunk[0] += 1

            # ---- identity initial state (fp16) ----
            sid = idpool.tile([128, FH], f16)
            nc.vector.memset(sid[:], 0.0)
            for k in range(4):
                nc.vector.memset(
                    sid[k * 32 : (k + 1) * 32, k * BW : (k + 1) * BW], 1.0
                )

            s_prev = None
            for i in range(N_BODIES):
                issue_trig(i)
                dual = i >= split
                s_next = spool.tile([128, 2 * FH], f16, tag="state")
                wd = w_tile[:, i * 192 : i * 192 + 128]
                w2 = w_tile[:, i * 192 + 128 : i * 192 + 192]

                # sub-units: rel slot first (its deps are a body older)
                units = []
                for slot in [1, 0] if dual else [0]:
                    if i == 0 or (slot == 1 and i in resets):
                        rhs = sid[:]
                    else:
                        off = FH if (slot == 1 and i > split) else 0
                        rhs = s_prev[:, off : off + FH]
                    for lo in (0, SUB):
                        units.append((slot, rhs, lo))

                # phase A: state-transform matmuls (independent per sub)
                us = []
                for slot, rhs, lo in units:
                    U = upool.tile([128, SUB], mybir.dt.float32, tag="u")
                    sl = slice(lo, lo + SUB)
                    nc.tensor.matmul(U[:], wd, rhs[:, sl], start=True, stop=True)
                    us.append(U)

                # phase B: trig mixes (DVE)
                tb = (
                    trig[:, i * BW : (i + 1) * BW]
                    .unsqueeze(1)
                    .broadcast_to([128, 2, BW])
                )
                pqs = []
                for (slot, rhs, lo), U in zip(units, us):
                    pq = pqpool.tile([128, SUB], f16, tag="pq")
                    nc.vector.tensor_mul(
                        pq[:].rearrange("p (r b) -> p r b", b=BW),
                        U[:].rearrange("p (r b) -> p r b", b=BW),
                        tb,
                    )
                    pqs.append(pq)

                # phase C: u23 + block-sum matmuls (X allocated late so its
                # PSUM bank is held only from here to the copy)
                xs = []
                for (slot, rhs, lo), pq in zip(units, pqs):
                    X = xpool.tile([128, SUB], mybir.dt.float32, tag="x")
                    sl = slice(lo, lo + SUB)
                    nc.tensor.matmul(
                        X[64:128, :], w2, rhs[:, sl], start=True, stop=True
                    )
                    nc.tensor.matmul(
                        X[0:64, :], wsum[:], pq[:], start=True, stop=True
                    )
                    xs.append(X)

                # phase D: PSUM -> fp16 state copies (ACT)
                for (slot, rhs, lo), X in zip(units, xs):
                    nc.scalar.copy(
                        s_next[:, slot * FH + lo : slot * FH + lo + SUB], X[:]
                    )

                # outputs (one DMA per slot)
                if dual:
                    nc.sync.dma_start(
                        orel_d[i - split, :, :], s_next[:, FH : 2 * FH]
                    )
                nc.sync.dma_start(oabs_d[i, :, :], s_next[:, 0:FH])
                s_prev = s_next

    nc.compile()
    return nc, split


def kernel(theta, offsets, reset_mask):
    theta = np.asarray(theta, dtype=np.float32)
    offsets = np.asarray(offsets, dtype=np.float32)
    reset_mask = np.asarray(reset_mask)
    assert theta.shape == (BATCH, N_BODIES)
    assert bool(reset_mask[0]), "chain must reset at body 0"
    resets = tuple(int(i) for i in np.flatnonzero(reset_mask) if i > 0)

    from concourse.bass_utils import run_bass_kernel_spmd

    key = resets
    if key not in _cache:
        _cache[key] = _build_program(resets)
    nc, split = _cache[key]

    # block-sum lhsT: col0 = PQ0 + PQ2, col1 = PQ1 + PQ3
    W_sum = np.zeros((128, 64), np.float16)
    gidx = np.arange(G)
    for q, j in [(0, 0), (2, 0), (1, 1), (3, 1)]:
        W_sum[q * G + gidx, j * G + gidx] = 1.0
    # per body: lhsT blocks for [u0,u1,u1,u0] (128 cols) and [u2,u3] (64 cols)
    W_all = np.zeros((128, N_BODIES * 192), np.float16)
    for i in range(N_BODIES):
        O = offsets[i]
        for k in range(4):
            for mb, j in enumerate([0, 1, 1, 0]):
                W_all[k * G + gidx, i * 192 + mb * G + gidx] = O[k, j]
            for mb, j in enumerate([2, 3]):
                W_all[k * G + gidx, i * 192 + 128 + mb * G + gidx] = O[k, j]

    # range-reduced replicated theta: [128, BC]; free layout (i, bw).
    # cos q-blocks (0,1): th - 2*pi*round(th/2pi + 1/4)  (in [-3pi/2, pi/2],
    # so +pi/2 sin bias stays within the LUT domain); sin q-blocks (2,3):
    # th - 2*pi*round(th/2pi)  (in [-pi, pi]).
    in_maps = []
    for c in range(N_CORES):
        thc = theta[c * BC : (c + 1) * BC]  # [8192, 32]
        th_g = np.ascontiguousarray(
            thc.reshape(G, BW, N_BODIES).transpose(0, 2, 1).reshape(G, N_BODIES * BW)
        ).astype(np.float64)  # [32, 8192] laid out (i, bw)
        th_c = th_g - 2 * np.pi * np.round(th_g / (2 * np.pi) + 0.25)
        th_s = th_g - 2 * np.pi * np.round(th_g / (2 * np.pi))
        threp = np.concatenate(
            [th_c, th_c, th_s, th_s], axis=0
        ).astype(np.float32)  # [128, 8192]
        in_maps.append({"threp": threp, "wall": W_all, "wsum": W_sum})

    out = run_bass_kernel_spmd(nc, in_maps, core_ids=list(range(N_CORES)))
    kernel.last_exec_ns = out.exec_time_ns
    kernel.last_results = out

    def decode(arr):
        # [nb, 128, FH] -> [nb, BC, 4, 4]: p=(k,g), f=(r,bw)
        nb = arr.shape[0]
        a = arr.astype(np.float32).reshape(nb, 4, G, 4, BW)  # i, k, g, r, bw
        return np.ascontiguousarray(
            a.transpose(0, 2, 4, 3, 1).reshape(nb, BC, 4, 4)
        )

    abs_full = np.empty((N_BODIES, BATCH, 4, 4), np.float32)
    rel_full = np.empty((N_BODIES, BATCH, 4, 4), np.float32)
    for c in range(N_CORES):
        res = out.results[c]
        bsl = slice(c * BC, (c + 1) * BC)
        abs_full[:, bsl] = decode(res["oabs"])
        rel_full[split:, bsl] = decode(res["orel"])
    rel_full[:split] = abs_full[:split]
    return abs_full, rel_full


kernel.last_exec_ns = None
kernel.last_results = None


# revision 16
# speedup vs baseline: 1.0198x; 1.0198x over previous
"""ChainKinematics Trainium2 kernel (8-core data-parallel), v5.

Math per batch element b:
  T_curr_i = offsets[i] @ Rz(theta[b, i])
  abs_i = abs_{i-1} @ T_curr_i           (abs_{-1} = I)
  rel_i = reset_i ? T_curr_i : rel_{i-1} @ T_curr_i

Layout (per core, 8192 batch elements, fp16 state):
  State S[k*32+g, r*256+bw] = A[g*256+bw, r, k].  Every chain slot is
  r-split into two independent 512-wide sub-chains (r in {0,1} / {2,3}).
  Per sub:
    U  = wd_i^T @ S            (PE; m-blocks [u0,u1,u1,u0], PSUM f32)
    X[64:128] = w2_i^T @ S     (PE; m-blocks [u2,u3] at partition base 64)
    pq = U * trig_i            (DVE; trig q-blocks [c,c,s,-s]; fp16 SBUF)
    X[0:64] = wsum^T @ pq      (PE; block sum -> [col0, col1])
    s_next = copy(X)           (ACT; one f32 PSUM -> fp16 SBUF copy)
  The theta input ships range-reduced per q-block (cos blocks use the
  +0.25-turn-shifted reduction), so trig is a single ACT Sin per chunk
  with per-partition scale [1,1,1,-1] / bias [pi/2, pi/2, 0, 0].
  All input DMAs are issued upfront (theta chunk 0 first) so the SP DMA
  queue never head-of-line blocks chain-dependent input loads.  For dual
  bodies the rel slot (whose deps are a full body older) is issued before
  the abs slot on every engine queue.
"""

import sys

sys.path.insert(0, "/opt/trn_rl_repo")

import numpy as np

N_BODIES = 32
BATCH = 65536
N_CORES = 8
BC = BATCH // N_CORES  # 8192 per core
G = 32  # batch groups (partition blocks)
BW = BC // G  # 256 batch per group
FH = 4 * BW  # 1024: free size of one chain-slot (r, bw)
SUB = FH // 2  # 512: r-split sub-slot
PI = float(np.pi)

# trig sin chunks over the (i, bw) free dim and the body index before
# which each chunk is issued
TRIG_BOUNDS = [0, 256, 512, 1024, 2048, 3072, 4096, 5120, 6144, 7168, 8192]
TRIG_ISSUE_BODY = [0, 0, 0, 1, 3, 5, 7, 9, 11, 13]
# upfront input DMA chunks for threp
TH_DMA_BOUNDS = [0, 256, 2048, 8192]

_cache = {}


def _build_program(resets):
    from concourse import bass, mybir, tile, bacc

    f32 = mybir.dt.float32
    f16 = mybir.dt.float16

    split = resets[0] if resets else N_BODIES  # first dual body

    nc = bacc.Bacc(None, target_bir_lowering=False, debug=False)
    threp_d = nc.dram_tensor("threp", [128, BC], f32, kind="ExternalInput")
    wall_d = nc.dram_tensor("wall", [128, N_BODIES * 192], f16, kind="ExternalInput")
    wsum_d = nc.dram_tensor("wsum", [128, 64], f16, kind="ExternalInput")
    oabs_d = nc.dram_tensor("oabs", [N_BODIES, 128, FH], f16, kind="ExternalOutput")
    orel_d = nc.dram_tensor(
        "orel", [N_BODIES - split, 128, FH], f16, kind="ExternalOutput"
    )

    with tile.TileContext(nc) as tc:
        with (
            tc.tile_pool(name="wpool", bufs=1) as wpool,
            tc.tile_pool(name="trigpool", bufs=1) as trigpool,
            tc.tile_pool(name="cpool", bufs=1) as cpool,
            tc.tile_pool(name="spool", bufs=4) as spool,
            tc.tile_pool(name="idpool", bufs=1) as idpool,
            tc.tile_pool(name="pqpool", bufs=6) as pqpool,
            tc.tile_pool(name="upool", bufs=4, space=bass.MemorySpace.PSUM) as upool,
            tc.tile_pool(name="xpool", bufs=4, space=bass.MemorySpace.PSUM) as xpool,
        ):
            # ---- upfront input DMAs (no waits; body-0 deps first) ----
            threp = trigpool.tile([128, BC], f32)
            w_tile = wpool.tile([128, N_BODIES * 192], f16)
            wsum = wpool.tile([128, 64], f16)
            nc.sync.dma_start(
                threp[:, 0 : TH_DMA_BOUNDS[1]], threp_d[:, 0 : TH_DMA_BOUNDS[1]]
            )
            nc.sync.dma_start(w_tile[:, 0 : split * 192], wall_d[:, 0 : split * 192])
            nc.sync.dma_start(wsum[:], wsum_d[:])
            for lo, hi in zip(TH_DMA_BOUNDS[1:-1], TH_DMA_BOUNDS[2:]):
                nc.sync.dma_start(threp[:, lo:hi], threp_d[:, lo:hi])
            nc.sync.dma_start(w_tile[:, split * 192 :], wall_d[:, split * 192 :])

            # ---- per-partition sin constants ----
            scl = cpool.tile([128, 1], f32)  # sin scale: [1,1,1,-1]
            nc.vector.memset(scl[0:96, :], 1.0)
            nc.vector.memset(scl[96:128, :], -1.0)
            bias = cpool.tile([128, 1], f32)  # sin bias: [pi/2, pi/2, 0, 0]
            nc.vector.memset(bias[0:64, :], float(PI / 2))
            nc.vector.memset(bias[64:128, :], 0.0)

            trig = trigpool.tile([128, BC], f16)

            n_chunks = len(TRIG_BOUNDS) - 1
            next_chunk = [0]

            def issue_trig(body):
                while (
                    next_chunk[0] < n_chunks
                    and TRIG_ISSUE_BODY[next_chunk[0]] <= body
                ):
                    k = next_chunk[0]
                    lo, hi = TRIG_BOUNDS[k], TRIG_BOUNDS[k + 1]
                    nc.scalar.activation(
                        trig[:, lo:hi], threp[:, lo:hi],
                        mybir.ActivationFunctionType.Sin,
                        bias=bias[:, 0:1], scale=scl[:, 0:1],
                    )
                    next_chunk[0] += 1

            # ---- identity initial state (fp16) ----
            sid = idpool.tile([128, FH], f16)
            nc.vector.memset(sid[:], 0.0)
            for k in range(4):
                nc.vector.memset(
                    sid[k * 32 : (k + 1) * 32, k * BW : (k + 1) * BW], 1.0
                )

            s_prev = None
            for i in range(N_BODIES):
                issue_trig(i)
                dual = i >= split
                s_next = spool.tile([128, 2 * FH], f16, tag="state")
                wd = w_tile[:, i * 192 : i * 192 + 128]
                w2 = w_tile[:, i * 192 + 128 : i * 192 + 192]

                # sub-units: rel slot first (its deps are a body older)
                units = []
                for slot in [1, 0] if dual else [0]:
                    if i == 0 or (slot == 1 and i in resets):
                        rhs = sid[:]
                    else:
                        off = FH if (slot == 1 and i > split) else 0
                        rhs = s_prev[:, off : off + FH]
                    for lo in (0, SUB):
                        units.append((slot, rhs, lo))

                # phase A: state-transform matmuls (independent per sub)
                us = []
                for slot, rhs, lo in units:
                    U = upool.tile([128, SUB], mybir.dt.float32, tag="u")
                    sl = slice(lo, lo + SUB)
                    nc.tensor.matmul(U[:], wd, rhs[:, sl], start=True, stop=True)
                    us.append(U)

                # phase B: trig mixes (DVE)
                tb = (
                    trig[:, i * BW : (i + 1) * BW]
                    .unsqueeze(1)
                    .broadcast_to([128, 2, BW])
                )
                pqs = []
                for (slot, rhs, lo), U in zip(units, us):
                    pq = pqpool.tile([128, SUB], f16, tag="pq")
                    nc.vector.tensor_mul(
                        pq[:].rearrange("p (r b) -> p r b", b=BW),
                        U[:].rearrange("p (r b) -> p r b", b=BW),
                        tb,
                    )
                    pqs.append(pq)

                # phase C: u23 + block-sum matmuls (X allocated late so its
                # PSUM bank is held only from here to the copy)
                xs = []
                for slot, rhs, lo in units:
                    X = xpool.tile([128, SUB], mybir.dt.float32, tag="x")
                    sl = slice(lo, lo + SUB)
                    nc.tensor.matmul(
                        X[64:128, :], w2, rhs[:, sl], start=True, stop=True
                    )
                    xs.append(X)
                for (slot, rhs, lo), X, pq in zip(units, xs, pqs):
                    nc.tensor.matmul(
                        X[0:64, :], wsum[:], pq[:], start=True, stop=True
                    )

                # phase D: PSUM -> fp16 state copies (ACT)
                for (slot, rhs, lo), X in zip(units, xs):
                    nc.scalar.copy(
                        s_next[:, slot * FH + lo : slot * FH + lo + SUB], X[:]
                    )

                # outputs (one DMA per slot)
                if dual:
                    nc.sync.dma_start(
                        orel_d[i - split, :, :], s_next[:, FH : 2 * FH]
                    )
                nc.sync.dma_start(oabs_d[i, :, :], s_next[:, 0:FH])
                s_prev = s_next

    nc.compile()
    return nc, split


def kernel(theta, offsets, reset_mask):
    theta = np.asarray(theta, dtype=np.float32)
    offsets = np.asarray(offsets, dtype=np.float32)
    reset_mask = np.asarray(reset_mask)
    assert theta.shape == (BATCH, N_BODIES)
    assert bool(reset_mask[0]), "chain must reset at body 0"
    resets = tuple(int(i) for i in np.flatnonzero(reset_mask) if i > 0)

    from concourse.bass_utils import run_bass_kernel_spmd

    key = resets
    if key not in _cache:
        _cache[key] = _build_program(resets)
    nc, split = _cache[key]

    # block-sum lhsT: col0 = PQ0 + PQ2, col1 = PQ1 + PQ3
    W_sum = np.zeros((128, 64), np.float16)
    gidx = np.arange(G)
    for q, j in [(0, 0), (2, 0), (1, 1), (3, 1)]:
        W_sum[q * G + gidx, j * G + gidx] = 1.0
    # per body: lhsT blocks for [u0,u1,u1,u0] (128 cols) and [u2,u3] (64 cols)
    W_all = np.zeros((128, N_BODIES * 192), np.float16)
    for i in range(N_BODIES):
        O = offsets[i]
        for k in range(4):
            for mb, j in enumerate([0, 1, 1, 0]):
                W_all[k * G + gidx, i * 192 + mb * G + gidx] = O[k, j]
            for mb, j in enumerate([2, 3]):
                W_all[k * G + gidx, i * 192 + 128 + mb * G + gidx] = O[k, j]

    # range-reduced replicated theta: [128, BC]; free layout (i, bw).
    # cos q-blocks (0,1): th - 2*pi*round(th/2pi + 1/4)  (in [-3pi/2, pi/2],
    # so +pi/2 sin bias stays within the LUT domain); sin q-blocks (2,3):
    # th - 2*pi*round(th/2pi)  (in [-pi, pi]).
    in_maps = []
    for c in range(N_CORES):
        thc = theta[c * BC : (c + 1) * BC]  # [8192, 32]
        th_g = np.ascontiguousarray(
            thc.reshape(G, BW, N_BODIES).transpose(0, 2, 1).reshape(G, N_BODIES * BW)
        ).astype(np.float64)  # [32, 8192] laid out (i, bw)
        th_c = th_g - 2 * np.pi * np.round(th_g / (2 * np.pi) + 0.25)
        th_s = th_g - 2 * np.pi * np.round(th_g / (2 * np.pi))
        threp = np.concatenate(
            [th_c, th_c, th_s, th_s], axis=0
        ).astype(np.float32)  # [128, 8192]
        in_maps.append({"threp": threp, "wall": W_all, "wsum": W_sum})

    out = run_bass_kernel_spmd(nc, in_maps, core_ids=list(range(N_CORES)))
    kernel.last_exec_ns = out.exec_time_ns
    kernel.last_results = out

    def decode(arr):
        # [nb, 128, FH] -> [nb, BC, 4, 4]: p=(k,g), f=(r,bw)
        nb = arr.shape[0]
        a = arr.astype(np.float32).reshape(nb, 4, G, 4, BW)  # i, k, g, r, bw
        return np.ascontiguousarray(
            a.transpose(0, 2, 4, 3, 1).reshape(nb, BC, 4, 4)
        )

    abs_full = np.empty((N_BODIES, BATCH, 4, 4), np.float32)
    rel_full = np.empty((N_BODIES, BATCH, 4, 4), np.float32)
    for c in range(N_CORES):
        res = out.results[c]
        bsl = slice(c * BC, (c + 1) * BC)
        abs_full[:, bsl] = decode(res["oabs"])
        rel_full[split:, bsl] = decode(res["orel"])
    rel_full[:split] = abs_full[:split]
    return abs_full, rel_full


kernel.last_exec_ns = None
kernel.last_results = None


# revision 17
# speedup vs baseline: 1.0821x; 1.0611x over previous
"""ChainKinematics Trainium2 kernel (8-core data-parallel), v5.

Math per batch element b:
  T_curr_i = offsets[i] @ Rz(theta[b, i])
  abs_i = abs_{i-1} @ T_curr_i           (abs_{-1} = I)
  rel_i = reset_i ? T_curr_i : rel_{i-1} @ T_curr_i

Layout (per core, 8192 batch elements, fp16 state):
  State S[k*32+g, r*256+bw] = A[g*256+bw, r, k].  Every chain slot is
  r-split into two independent 512-wide sub-chains (r in {0,1} / {2,3}).
  Per sub:
    U  = wd_i^T @ S            (PE; m-blocks [u0,u1,u1,u0], PSUM f32)
    X[64:128] = w2_i^T @ S     (PE; m-blocks [u2,u3] at partition base 64)
    pq = U * trig_i            (DVE; trig q-blocks [c,c,s,-s]; fp16 SBUF)
    X[0:64] = wsum^T @ pq      (PE; block sum -> [col0, col1])
    s_next = copy(X)           (ACT; one f32 PSUM -> fp16 SBUF copy)
  The theta input ships range-reduced per q-block (cos blocks use the
  +0.25-turn-shifted reduction), so trig is a single ACT Sin per chunk
  with per-partition scale [1,1,1,-1] / bias [pi/2, pi/2, 0, 0].
  All input DMAs are issued upfront (theta chunk 0 first) so the SP DMA
  queue never head-of-line blocks chain-dependent input loads.  For dual
  bodies the rel slot (whose deps are a full body older) is issued before
  the abs slot on every engine queue.
"""

import sys

sys.path.insert(0, "/opt/trn_rl_repo")

import numpy as np

N_BODIES = 32
BATCH = 65536
N_CORES = 8
BC = BATCH // N_CORES  # 8192 per core
G = 32  # batch groups (partition blocks)
BW = BC // G  # 256 batch per group
FH = 4 * BW  # 1024: free size of one chain-slot (r, bw)
SUB = FH // 2  # 512: r-split sub-slot
PI = float(np.pi)

# trig sin chunks over the (i, bw) free dim and the body index before
# which each chunk is issued
TRIG_BOUNDS = [0, 256, 512, 1024, 2048, 3072, 4096, 5120, 6144, 7168, 8192]
TRIG_ISSUE_BODY = [0, 0, 0, 1, 2, 3, 4, 5, 6, 7]
# upfront input DMA chunks for threp
TH_DMA_BOUNDS = [0, 256, 2048, 8192]

_cache = {}


def _build_program(resets):
    from concourse import bass, mybir, tile, bacc

    f32 = mybir.dt.float32
    f16 = mybir.dt.float16

    split = resets[0] if resets else N_BODIES  # first dual body

    nc = bacc.Bacc(None, target_bir_lowering=False, debug=False)
    threp_d = nc.dram_tensor("threp", [128, BC], f16, kind="ExternalInput")
    wall_d = nc.dram_tensor("wall", [128, N_BODIES * 192], f16, kind="ExternalInput")
    wsum_d = nc.dram_tensor("wsum", [128, 64], f16, kind="ExternalInput")
    oabs_d = nc.dram_tensor("oabs", [N_BODIES, 128, FH], f16, kind="ExternalOutput")
    orel_d = nc.dram_tensor(
        "orel", [N_BODIES - split, 128, FH], f16, kind="ExternalOutput"
    )

    with tile.TileContext(nc) as tc:
        with (
            tc.tile_pool(name="wpool", bufs=1) as wpool,
            tc.tile_pool(name="trigpool", bufs=1) as trigpool,
            tc.tile_pool(name="cpool", bufs=1) as cpool,
            tc.tile_pool(name="spool", bufs=4) as spool,
            tc.tile_pool(name="idpool", bufs=1) as idpool,
            tc.tile_pool(name="pqpool", bufs=6) as pqpool,
            tc.tile_pool(name="upool", bufs=4, space=bass.MemorySpace.PSUM) as upool,
            tc.tile_pool(name="xpool", bufs=4, space=bass.MemorySpace.PSUM) as xpool,
        ):
            # ---- upfront input DMAs (no waits; body-0 deps first) ----
            threp = trigpool.tile([128, BC], f16)
            w_tile = wpool.tile([128, N_BODIES * 192], f16)
            wsum = wpool.tile([128, 64], f16)
            nc.sync.dma_start(
                threp[:, 0 : TH_DMA_BOUNDS[1]], threp_d[:, 0 : TH_DMA_BOUNDS[1]]
            )
            nc.sync.dma_start(w_tile[:, 0 : split * 192], wall_d[:, 0 : split * 192])
            nc.sync.dma_start(wsum[:], wsum_d[:])
            for lo, hi in zip(TH_DMA_BOUNDS[1:-1], TH_DMA_BOUNDS[2:]):
                nc.sync.dma_start(threp[:, lo:hi], threp_d[:, lo:hi])
            nc.sync.dma_start(w_tile[:, split * 192 :], wall_d[:, split * 192 :])

            # ---- per-partition sin constants ----
            scl = cpool.tile([128, 1], f32)  # sin scale: [1,1,1,-1]
            nc.vector.memset(scl[0:96, :], 1.0)
            nc.vector.memset(scl[96:128, :], -1.0)
            bias = cpool.tile([128, 1], f32)  # sin bias: [pi/2, pi/2, 0, 0]
            nc.vector.memset(bias[0:64, :], float(PI / 2))
            nc.vector.memset(bias[64:128, :], 0.0)

            trig = trigpool.tile([128, BC], f16)

            n_chunks = len(TRIG_BOUNDS) - 1
            next_chunk = [0]

            def issue_trig(body):
                while (
                    next_chunk[0] < n_chunks
                    and TRIG_ISSUE_BODY[next_chunk[0]] <= body
                ):
                    k = next_chunk[0]
                    lo, hi = TRIG_BOUNDS[k], TRIG_BOUNDS[k + 1]
                    nc.scalar.activation(
                        trig[:, lo:hi], threp[:, lo:hi],
                        mybir.ActivationFunctionType.Sin,
                        bias=bias[:, 0:1], scale=scl[:, 0:1],
                    )
                    next_chunk[0] += 1

            # ---- identity initial state (fp16) ----
            sid = idpool.tile([128, FH], f16)
            nc.vector.memset(sid[:], 0.0)
            for k in range(4):
                nc.vector.memset(
                    sid[k * 32 : (k + 1) * 32, k * BW : (k + 1) * BW], 1.0
                )

            s_prev = None
            for i in range(N_BODIES):
                issue_trig(i)
                dual = i >= split
                s_next = spool.tile([128, 2 * FH], f16, tag="state")
                wd = w_tile[:, i * 192 : i * 192 + 128]
                w2 = w_tile[:, i * 192 + 128 : i * 192 + 192]

                # sub-units: rel slot first (its deps are a body older)
                units = []
                for slot in [1, 0] if dual else [0]:
                    if i == 0 or (slot == 1 and i in resets):
                        rhs = sid[:]
                    else:
                        off = FH if (slot == 1 and i > split) else 0
                        rhs = s_prev[:, off : off + FH]
                    for lo in (0, SUB):
                        units.append((slot, rhs, lo))

                # phase A: state-transform matmuls (independent per sub)
                us = []
                for slot, rhs, lo in units:
                    U = upool.tile([128, SUB], mybir.dt.float32, tag="u")
                    sl = slice(lo, lo + SUB)
                    nc.tensor.matmul(U[:], wd, rhs[:, sl], start=True, stop=True)
                    us.append(U)

                # phase B: trig mixes (DVE)
                tb = (
                    trig[:, i * BW : (i + 1) * BW]
                    .unsqueeze(1)
                    .broadcast_to([128, 2, BW])
                )
                pqs = []
                for (slot, rhs, lo), U in zip(units, us):
                    pq = pqpool.tile([128, SUB], f16, tag="pq")
                    nc.vector.tensor_mul(
                        pq[:].rearrange("p (r b) -> p r b", b=BW),
                        U[:].rearrange("p (r b) -> p r b", b=BW),
                        tb,
                    )
                    pqs.append(pq)

                # phase C: u23 + block-sum matmuls (X allocated late so its
                # PSUM bank is held only from here to the copy)
                xs = []
                for slot, rhs, lo in units:
                    X = xpool.tile([128, SUB], mybir.dt.float32, tag="x")
                    sl = slice(lo, lo + SUB)
                    nc.tensor.matmul(
                        X[64:128, :], w2, rhs[:, sl], start=True, stop=True
                    )
                    xs.append(X)
                for (slot, rhs, lo), X, pq in zip(units, xs, pqs):
                    nc.tensor.matmul(
                        X[0:64, :], wsum[:], pq[:], start=True, stop=True
                    )

                # phase D: PSUM -> fp16 state copies (ACT)
                for (slot, rhs, lo), X in zip(units, xs):
                    nc.scalar.copy(
                        s_next[:, slot * FH + lo : slot * FH + lo + SUB], X[:]
                    )

                # outputs (one DMA per slot)
                if dual:
                    nc.sync.dma_start(
                        orel_d[i - split, :, :], s_next[:, FH : 2 * FH]
                    )
                nc.sync.dma_start(oabs_d[i, :, :], s_next[:, 0:FH])
                s_prev = s_next

    nc.compile()
    return nc, split


def kernel(theta, offsets, reset_mask):
    theta = np.asarray(theta, dtype=np.float32)
    offsets = np.asarray(offsets, dtype=np.float32)
    reset_mask = np.asarray(reset_mask)
    assert theta.shape == (BATCH, N_BODIES)
    assert bool(reset_mask[0]), "chain must reset at body 0"
    resets = tuple(int(i) for i in np.flatnonzero(reset_mask) if i > 0)

    from concourse.bass_utils import run_bass_kernel_spmd

    key = resets
    if key not in _cache:
        _cache[key] = _build_program(resets)
    nc, split = _cache[key]

    # block-sum lhsT: col0 = PQ0 + PQ2, col1 = PQ1 + PQ3
    W_sum = np.zeros((128, 64), np.float16)
    gidx = np.arange(G)
    for q, j in [(0, 0), (2, 0), (1, 1), (3, 1)]:
        W_sum[q * G + gidx, j * G + gidx] = 1.0
    # per body: lhsT blocks for [u0,u1,u1,u0] (128 cols) and [u2,u3] (64 cols)
    W_all = np.zeros((128, N_BODIES * 192), np.float16)
    for i in range(N_BODIES):
        O = offsets[i]
        for k in range(4):
            for mb, j in enumerate([0, 1, 1, 0]):
                W_all[k * G + gidx, i * 192 + mb * G + gidx] = O[k, j]
            for mb, j in enumerate([2, 3]):
                W_all[k * G + gidx, i * 192 + 128 + mb * G + gidx] = O[k, j]

    # range-reduced replicated theta: [128, BC]; free layout (i, bw).
    # cos q-blocks (0,1): th - 2*pi*round(th/2pi + 1/4)  (in [-3pi/2, pi/2],
    # so +pi/2 sin bias stays within the LUT domain); sin q-blocks (2,3):
    # th - 2*pi*round(th/2pi)  (in [-pi, pi]).
    in_maps = []
    for c in range(N_CORES):
        thc = theta[c * BC : (c + 1) * BC]  # [8192, 32]
        th_g = np.ascontiguousarray(
            thc.reshape(G, BW, N_BODIES).transpose(0, 2, 1).reshape(G, N_BODIES * BW)
        ).astype(np.float64)  # [32, 8192] laid out (i, bw)
        th_c = th_g - 2 * np.pi * np.round(th_g / (2 * np.pi) + 0.25)
        th_s = th_g - 2 * np.pi * np.round(th_g / (2 * np.pi))
        threp = np.concatenate(
            [th_c, th_c, th_s, th_s], axis=0
        ).astype(np.float16)  # [128, 8192]
        in_maps.append({"threp": threp, "wall": W_all, "wsum": W_sum})

    out = run_bass_kernel_spmd(nc, in_maps, core_ids=list(range(N_CORES)))
    kernel.last_exec_ns = out.exec_time_ns
    kernel.last_results = out

    def decode(arr):
        # [nb, 128, FH] -> [nb, BC, 4, 4]: p=(k,g), f=(r,bw)
        nb = arr.shape[0]
        a = arr.astype(np.float32).reshape(nb, 4, G, 4, BW)  # i, k, g, r, bw
        return np.ascontiguousarray(
            a.transpose(0, 2, 4, 3, 1).reshape(nb, BC, 4, 4)
        )

    abs_full = np.empty((N_BODIES, BATCH, 4, 4), np.float32)
    rel_full = np.empty((N_BODIES, BATCH, 4, 4), np.float32)
    for c in range(N_CORES):
        res = out.results[c]
        bsl = slice(c * BC, (c + 1) * BC)
        abs_full[:, bsl] = decode(res["oabs"])
        rel_full[split:, bsl] = decode(res["orel"])
    rel_full[:split] = abs_full[:split]
    return abs_full, rel_full


kernel.last_exec_ns = None
kernel.last_results = None
